# revision 1
# baseline (speedup 1.0000x reference)
"""Trainium2 Bass kernel for nn_Model_11888469475981 (pooling).

Reference semantics (per sample n, channel c):
  x_p = zeropad3d(x, W:(1,2), H:(1,1), D:(0,1))           # (17, 66, 259)
  rows = x_p rows along W (each length Wp=259), K=3 S=2 maxpool w/ indices,
  softsign, max-unpool scatter, add x_p, mean over padded D (17).

Key restructure (exact, no gather/scatter):
  For a padded row A[0..258], position w receives softsign(A[w]) iff some
  window picks w as its (first-occurrence) argmax. With L[w] = [A[w] > A[w-1]]
  and e1[m] = [A[2m] >= A[2m+2]]:
    odd w=2m+1 : mask = L[2m+1] * (1 - L[2m+2])
    even w=2m  : mask = max( (1-L[2m+1])*e1[m],  (1-e1[m-1])*L[2m] )
  fused[w] = A[w] * (1 + mask[w] * 1/(1+|A[w]|))
  out[h, w] = (1/17) * sum_d fused[d, h, w]   (padded D slab and padded H/W
  rows are exactly zero and are written as zeros / via the 1/17 weight).

Layout per core (1 sample): per channel c, one SBUF tile [128, 8*264]:
  partition p = d*8 + hg  (d in 0..15, hg = h//8), free = hs-slot (h%8) * 264.
  Slot: [2 guard][259 padded-W cols][3 guard], real x at cols 3..258.
  Depth-mean via PE matmul with lhsT W8[p, m] = (1/17)*[p%8 == m] -> psum[8,264].
"""

import numpy as np

import concourse.bass as bass
import concourse.mybir as mybir
from concourse import bacc
from concourse.tile import TileContext
from concourse.bass_utils import run_bass_kernel_spmd

N_CORES = 8
C, D, H, W = 32, 16, 64, 256
HP, WP = 66, 259
SLOT = 264
NS = 8              # h-subslots per partition
FREE = NS * SLOT
DSLOT = 132         # dense (per-window-index m) slot width
DFREE = NS * DSLOT
F32 = mybir.dt.float32
Alu = mybir.AluOpType
Act = mybir.ActivationFunctionType


def _fullw(t, c0, cnt):
    return t[:].rearrange("p (s w) -> p s w", s=NS)[:, :, c0:c0 + cnt]


def _dense(t, c0, cnt):
    return t[:].rearrange("p (s w) -> p s w", s=NS)[:, :, c0:c0 + cnt]


def _v2(t):
    return t[:].rearrange("p (s w2 two) -> p s w2 two", s=NS, two=2)


def _ev(t, mshift, cnt):
    # even padded-w columns: col = 2 + 2*(m + mshift), m in [0, cnt)
    return _v2(t)[:, :, 1 + mshift:1 + mshift + cnt, 0]


def _od(t, mshift, cnt):
    # odd padded-w columns: col = 3 + 2*(m + mshift), m in [0, cnt)
    return _v2(t)[:, :, 1 + mshift:1 + mshift + cnt, 1]


def build_nc():
    # Bacc: its finalize() runs the wait-splitting / legalization passes
    # (TRN2 allows at most 1 sync wait per instruction).
    nc = bacc.Bacc()
    x_ext = nc.declare_dram_parameter("x", [C, D, H, W], F32, isOutput=False)
    w8_ext = nc.declare_dram_parameter("w8", [128, 8], F32, isOutput=False)
    out_ext = nc.declare_dram_parameter("out", [C, HP, WP], F32, isOutput=True)

    with TileContext(nc) as tc:
        with tc.tile_pool(name="main", bufs=1) as pool, \
             tc.tile_pool(name="psum", bufs=2, space="PSUM") as psum_pool:
            a_ts = [pool.tile([128, FREE], F32, tag=f"a{i}", name=f"a{i}") for i in range(3)]
            f_ts = [pool.tile([128, FREE], F32, tag=f"fu{i}", name=f"fu{i}") for i in range(2)]
            m2_ts = [pool.tile([128, FREE], F32, tag=f"m2{i}", name=f"m2{i}") for i in range(2)]
            r_ts = [pool.tile([128, FREE], F32, tag=f"r{i}", name=f"r{i}") for i in range(2)]
            ab_t = pool.tile([128, FREE], F32, tag="abs", name="abs")
            ln_t = pool.tile([128, FREE], F32, tag="lnt", name="lnt")
            l_t = pool.tile([128, FREE], F32, tag="lcmp", name="lcmp")
            e1_t = pool.tile([128, DFREE], F32, tag="e1", name="e1")
            to_t = pool.tile([128, DFREE], F32, tag="todd", name="todd")
            fe_t = pool.tile([128, DFREE], F32, tag="fev", name="fev")
            le_t = pool.tile([128, DFREE], F32, tag="lev", name="lev")
            mk_t = pool.tile([128, DFREE], F32, tag="mask", name="mask")
            w8_t = pool.tile([128, 8], F32, tag="w8", name="w8")
            o_ts = [pool.tile([8, NS * WP], F32, tag=f"o{i}", name=f"o{i}")
                    for i in range(2)]
            z_t = pool.tile([32, 2 * WP], F32, tag="zrow", name="zrow")

            # one-time init: zero guards (and any never-written-but-read cols).
            # Memsets run on DVE so downstream DVE/PE consumers do not need an
            # extra cross-engine semaphore wait (walrus caps waits per inst).
            for t in a_ts + f_ts + m2_ts + r_ts:
                nc.vector.memset(t[:], 0.0)
            nc.vector.memset(l_t[:], 0.0)
            nc.vector.memset(e1_t[:], 0.0)
            nc.gpsimd.memset(z_t[:], 0.0)
            nc.sync.dma_start(out=w8_t[:], in_=w8_ext[:, :])

            # padded-H border rows (h'=0 and h'=65) for every channel: zeros
            nc.sync.dma_start(
                out=bass.AP(out_ext, 0, [[HP * WP, C], [65 * WP, 2], [1, WP]]),
                in_=z_t[:].rearrange("p (a w) -> p a w", w=WP),
            )

            for c in range(C):
                a_t = a_ts[c % 3]
                F_t = f_ts[c % 2]
                m2_t = m2_ts[c % 2]
                r_t = r_ts[c % 2]

                # load channel: rows r=(d*64+h) -> partition p=d*8+h//8, slot h%8
                av = a_t[:].rearrange("p (s w) -> p s w", s=NS)
                nc.sync.dma_start(
                    out=av[:, :, 3:259],
                    in_=bass.AP(
                        x_ext,
                        c * D * H * W,
                        [[2048, 128], [256, NS], [1, W]],
                    ),
                )

                # L[w] = A[w] > A[w-1], w=0..258 (cols 2..260)
                nc.vector.tensor_tensor(
                    _fullw(l_t, 2, 259), _fullw(a_t, 2, 259), _fullw(a_t, 1, 259),
                    Alu.is_gt)
                # e1[m] = A[2m] >= A[2m+2], m=0..129
                nc.vector.tensor_tensor(
                    _dense(e1_t, 2, 130), _ev(a_t, 0, 130), _ev(a_t, 1, 130),
                    Alu.is_ge)
                # odd mask: todd[m] = (L[2m+2]==0) * L[2m+1], m=0..128
                nc.vector.scalar_tensor_tensor(
                    _dense(to_t, 2, 129), _ev(l_t, 1, 129), 0.0, _od(l_t, 0, 129),
                    Alu.is_equal, Alu.mult)
                # even "first": fe[m] = (L[2m+1]==0) * e1[m], m=0..129
                nc.vector.scalar_tensor_tensor(
                    _dense(fe_t, 2, 130), _od(l_t, 0, 130), 0.0, _dense(e1_t, 2, 130),
                    Alu.is_equal, Alu.mult)
                # even "last": le[m] = (e1[m-1]==0) * L[2m], m=0..129
                nc.vector.scalar_tensor_tensor(
                    _dense(le_t, 2, 130), _dense(e1_t, 1, 130), 0.0, _ev(l_t, 0, 130),
                    Alu.is_equal, Alu.mult)
                # even mask = max(first, last)
                nc.vector.tensor_tensor(
                    _dense(mk_t, 2, 130), _dense(fe_t, 2, 130), _dense(le_t, 2, 130),
                    Alu.max)

                # softsign reciprocal on ACT: r = 1/(1+|A|) = sigmoid(-ln|A|).
                # Only real cols 3..258; r at pad cols stays 0 from the
                # one-time memset (m2 = mask*0 = 0 there, and A=0 -> F=0).
                nc.scalar.activation(_fullw(ab_t, 3, 256), _fullw(a_t, 3, 256),
                                     Act.Abs)
                nc.scalar.activation(_fullw(ln_t, 3, 256), _fullw(ab_t, 3, 256),
                                     Act.Ln)
                nc.scalar.activation(_fullw(r_t, 3, 256), _fullw(ln_t, 3, 256),
                                     Act.Sigmoid, scale=-1.0)

                # m2 = mask * r  (parity-split writes)
                nc.vector.tensor_tensor(
                    _od(m2_t, 0, 129), _dense(to_t, 2, 129), _od(r_t, 0, 129),
                    Alu.mult)
                nc.vector.tensor_tensor(
                    _ev(m2_t, 0, 130), _dense(mk_t, 2, 130), _ev(r_t, 0, 130),
                    Alu.mult)
                # fused = (m2 + 1) * A
                nc.vector.scalar_tensor_tensor(
                    _fullw(F_t, 2, 260), _fullw(m2_t, 2, 260), 1.0,
                    _fullw(a_t, 2, 260), Alu.add, Alu.mult)

                # depth-sum via PE: psum[hg, w] = sum_d F[(d,hg), w], then
                # ScalarE evacuates PSUM->SBUF applying the 1/17 mean scale.
                Fv = F_t[:].rearrange("p (s w) -> p s w", s=NS)
                osb = o_ts[c % 2]
                ov = osb[:].rearrange("p (s w) -> p s w", s=NS)
                for half in range(2):
                    ps = psum_pool.tile([8, 4 * 512], F32, tag="ps",
                                        name=f"ps_{c}_{half}")
                    psv = ps[:].rearrange("p (s w) -> p s w", s=4)
                    for k in range(4):
                        hs = half * 4 + k
                        nc.tensor.matmul(psv[:, k, 0:SLOT], w8_t[:, 0:8],
                                         Fv[:, hs, :], start=True, stop=True)
                    nc.scalar.mul(ov[:, 4 * half:4 * half + 4, :],
                                  psv[:, :, 2:261], 1.0 / 17.0)
                nc.sync.dma_start(
                    out=bass.AP(out_ext, (c * HP + 1) * WP,
                                [[8 * WP, 8], [WP, NS], [1, WP]]),
                    in_=ov[:, :, :],
                )
    nc.finalize()
    return nc


_CACHE: dict = {}


def _get_nc():
    if "nc" not in _CACHE:
        _CACHE["nc"] = build_nc()
    return _CACHE["nc"]


def make_in_maps(x: np.ndarray):
    w8 = np.zeros((128, 8), np.float32)
    w8[np.arange(128), np.arange(128) % 8] = 1.0
    return [
        {"x": np.ascontiguousarray(x[i]), "w8": w8}
        for i in range(N_CORES)
    ]


def kernel(**inputs) -> np.ndarray:
    x = np.ascontiguousarray(np.asarray(inputs["x"], dtype=np.float32))
    assert x.shape == (N_CORES, C, D, H, W), x.shape
    nc = _get_nc()
    res = run_bass_kernel_spmd(nc, make_in_maps(x), list(range(N_CORES)))
    return np.stack([res.results[i]["out"] for i in range(N_CORES)], axis=0)



# revision 6
# speedup vs baseline: 1.2096x; 1.2096x over previous
"""Trainium2 Bass kernel for nn_Model_11888469475981 (pooling).

Reference semantics (per sample n, channel c, row (d,h) along W):
  pad W by (1,2) -> row A[0..258]; K=3 S=2 maxpool w/ indices (L=129
  windows), softsign, max-unpool scatter, add padded input, mean over
  padded D (17 slabs, one all-zero).

Restructure (per padded row, half-grid m with ev[m]=A[2m], od[m]=A[2m+1]):
  Q[m]   = max(ev[m], od[m], ev[m+1])          window max, m=0..128
  G[m]   = min(Q[m], Q[m-1])                   (guards = +BIG)
  modd   = od >= Q     meven = ev >= G         selection masks
  SQ     = Q * (1/(1+|Q|));  SG = min(SQ, SQ[m-1])   (softsign monotone)
  ms_od  = modd*SQ;  ms_ev = meven*SG          masked softsign values
  out    = (1/17) * sum_d (A + ms)
Masks/values in bf16 (L2 err ~7e-3, gate 2e-2); softsign reciprocal on
the Activation engine (Reciprocal table, |err| ~1e-3, plenty here).

Layout per core (1 sample): per channel, tile [128, 8*SW]:
  partition p = d*8 + hb (d=0..15, hb=h//8), slot s = h%8.
  Parity slot layout (SW=264): cols 0..129 = ev[0..129], 130..258 =
  od[0..128], 259..263 junk. ev[0], ev[129], od[128] are W-pads (zero).
  ACT+GpSimd de-interleave the fp32 DMA tile into this bf16 layout; all
  DVE ops are then packed stride-1 (2x bf16 mode). PE sums over d with
  two chained bf16 matmuls per slot (selector weights); ACT evacuates
  PSUM with the 1/17 scale folded in, re-interleaving parities.
"""

import numpy as np
import ml_dtypes

import concourse.bass as bass
import concourse.mybir as mybir
from concourse import bacc
from concourse.tile import TileContext
from concourse.bass_utils import run_bass_kernel_spmd

N_CORES = 8
C, D, H, W = 32, 16, 64, 256
HP, WP = 66, 259
NS = 8            # h-subslots per partition
SW = 264          # parity slot width: 130 ev + 129 od + 5 junk
EV0, NEV = 0, 130
OD0, NOD = 130, 129
QW = 136          # Q/SQ slot: [guard][129][guard][junk]
F32 = mybir.dt.float32
BF16 = mybir.dt.bfloat16
Alu = mybir.AluOpType
Act = mybir.ActivationFunctionType
BIG = 1e30


def _s(t, w):
    return t[:].rearrange("p (s w) -> p s w", w=w)


def _act(nc, out, in_, func, scale=1.0, bias=0.0):
    # direct InstActivation emission (the nc.scalar.activation wrapper
    # refuses Reciprocal; its |err|~1e-3 is fine for softsign values)
    eng = nc.scalar
    ins = [eng.lower_ap(in_)]
    for arg in (bias, scale, 0.0):
        ins.append(mybir.ImmediateValue(dtype=mybir.dt.float32, value=arg))
    return eng.add_instruction(
        mybir.InstActivation(name=nc.get_next_instruction_name(),
                             func=func, ins=ins, outs=[eng.lower_ap(out)]))


def build_nc():
    nc = bacc.Bacc()
    x_ext = nc.declare_dram_parameter("x", [C, D, H, W], F32, isOutput=False)
    w8_ext = nc.declare_dram_parameter("w8", [128, 8], BF16, isOutput=False)
    out_ext = nc.declare_dram_parameter("out", [C, HP, WP], F32, isOutput=True)

    with TileContext(nc) as tc:
        with tc.tile_pool(name="main", bufs=1) as pool, \
             tc.tile_pool(name="psum", bufs=2, space="PSUM") as psum_pool:
            def tiles(nm, shape, dtype):
                return [pool.tile(shape, dtype, tag=f"{nm}{i}",
                                  name=f"{nm}{i}") for i in range(2)]

            a_ts = tiles("a", [128, NS * W], F32)
            ab_ts = tiles("ab", [128, NS * SW], BF16)
            p_ts = tiles("p", [128, NS * QW], BF16)
            q_ts = tiles("q", [128, NS * QW], BF16)
            g_ts = tiles("g", [128, NS * QW], BF16)
            aq_ts = tiles("aq", [128, NS * QW], BF16)
            rq_ts = tiles("rq", [128, NS * QW], BF16)
            sq_ts = tiles("sq", [128, NS * QW], BF16)
            sg_ts = tiles("sg", [128, NS * QW], BF16)
            mk_ts = tiles("mk", [128, NS * SW], BF16)
            ms_ts = tiles("ms", [128, NS * SW], BF16)
            # slot width 260 (not 259) so stride-2 parity views factorize
            o_ts = tiles("o", [8, NS * 260], F32)
            w8_t = pool.tile([128, 8], BF16, tag="w8", name="w8")
            z_t = pool.tile([32, 2 * WP], F32, tag="zrow", name="zrow")

            # one-time init: W-pad zeros + junk cols (real cols are fully
            # rewritten every channel; pads persist). +BIG guards in Q
            # tiles for G = min(Q, Q<<1); 1.0 guards in SQ (softsign(BIG)).
            for t in ab_ts + mk_ts + ms_ts:
                nc.vector.memset(t[:], 0.0)
            for t in q_ts:
                nc.vector.memset(t[:], BIG)
            for t in sq_ts:
                nc.vector.memset(t[:], 1.0)
            nc.gpsimd.memset(z_t[:], 0.0)
            nc.sync.dma_start(out=w8_t[:], in_=w8_ext[:, :])

            # padded-H border rows (h'=0 and h'=65) for every channel: zeros
            nc.sync.dma_start(
                out=bass.AP(out_ext, 0, [[HP * WP, C], [65 * WP, 2], [1, WP]]),
                in_=z_t[:].rearrange("p (a w) -> p a w", w=WP),
            )

            for c in range(C):
                a_t, ab_t, p_t, q_t, g_t = (t[c % 2] for t in
                                            (a_ts, ab_ts, p_ts, q_ts, g_ts))
                aq_t, rq_t, sq_t, sg_t = (t[c % 2] for t in
                                          (aq_ts, rq_ts, sq_ts, sg_ts))
                mk_t, ms_t, o_t = (t[c % 2] for t in (mk_ts, ms_ts, o_ts))

                # load channel: 2048 contiguous f32 per partition
                nc.sync.dma_start(
                    out=a_t[:],
                    in_=bass.AP(x_ext, c * D * H * W, [[2048, 128], [1, 2048]]),
                )

                av2 = a_t[:].rearrange("p (s w2 two) -> p s w2 two",
                                       s=NS, two=2)
                abv = _s(ab_t, SW)
                # de-interleave + cast:  ev[m]=x[2m-1] (m=1..128, odd reals,
                # on ACT);  od[m]=x[2m] (m=0..127, even reals, on GpSimd)
                nc.scalar.copy(abv[:, :, 1:129], av2[:, :, 0:128, 1])
                nc.gpsimd.tensor_copy(abv[:, :, OD0:OD0 + 128],
                                      av2[:, :, 0:128, 0])

                ev = abv[:, :, 0:129]        # ev[m], m=0..128
                evp = abv[:, :, 1:130]       # ev[m+1]
                od = abv[:, :, OD0:OD0 + NOD]
                pv = _s(p_t, QW)[:, :, 0:129]
                qv = _s(q_t, QW)             # Q[m] at col 1+m
                gv = _s(g_t, QW)[:, :, 0:130]
                q_ = qv[:, :, 1:130]

                # window max Q and neighbor-min G (DVE, bf16 2x)
                nc.vector.tensor_tensor(pv, ev, od, Alu.max)
                nc.vector.tensor_tensor(q_, pv, evp, Alu.max)
                nc.vector.tensor_tensor(
                    gv, qv[:, :, 1:131], qv[:, :, 0:130], Alu.min)

                # selection masks (DVE)
                mkv = _s(mk_t, SW)
                nc.vector.tensor_tensor(
                    mkv[:, :, OD0:OD0 + NOD], od, q_, Alu.is_ge)
                nc.vector.tensor_tensor(
                    mkv[:, :, EV0:EV0 + NEV], abv[:, :, 0:130], gv, Alu.is_ge)

                # softsign of window maxes: rQ = 1/(1+|Q|) on ACT tables,
                # SQ = Q*rQ, SG = min(SQ, SQ[m-1])  (softsign is monotone)
                aqv = _s(aq_t, QW)[:, :, 1:130]
                rqv = _s(rq_t, QW)[:, :, 1:130]
                sqv = _s(sq_t, QW)
                _act(nc, aqv, q_, Act.Abs)
                _act(nc, rqv, aqv, Act.Reciprocal, scale=1.0, bias=1.0)
                nc.vector.tensor_tensor(sqv[:, :, 1:130], q_, rqv, Alu.mult)
                sgv = _s(sg_t, QW)[:, :, 0:130]
                nc.vector.tensor_tensor(
                    sgv, sqv[:, :, 1:131], sqv[:, :, 0:130], Alu.min)

                # masked values: ms_od = modd*SQ (GpSimd), ms_ev = meven*SG
                msv = _s(ms_t, SW)
                nc.gpsimd.tensor_tensor(
                    msv[:, :, OD0:OD0 + NOD], mkv[:, :, OD0:OD0 + NOD],
                    sqv[:, :, 1:130], Alu.mult)
                nc.vector.tensor_tensor(
                    msv[:, :, EV0:EV0 + NEV], mkv[:, :, EV0:EV0 + NEV],
                    sgv, Alu.mult)

                # depth-sum via PE: psum += W8^T @ Abf + W8^T @ ms
                ov4 = o_t[:].rearrange("p (s w2 two) -> p s w2 two",
                                       s=NS, two=2)
                for half in range(2):
                    ps = psum_pool.tile([8, 4 * 512], F32, tag="ps",
                                        name=f"ps_{c}_{half}")
                    psv = ps[:].rearrange("p (s w) -> p s w", s=4)
                    for k in range(4):
                        hs = half * 4 + k
                        nc.tensor.matmul(psv[:, k, 0:SW], w8_t[:, 0:8],
                                         abv[:, hs, :], start=True, stop=False)
                        nc.tensor.matmul(psv[:, k, 0:SW], w8_t[:, 0:8],
                                         _s(ms_t, SW)[:, hs, :],
                                         start=False, stop=True)
                    # evacuate + 1/17 + re-interleave parities (ACT)
                    oh = ov4[:, 4 * half:4 * half + 4]
                    nc.scalar.mul(oh[:, :, 0:130, 0],
                                  psv[:, :, EV0:EV0 + NEV], 1.0 / 17.0)
                    nc.scalar.mul(oh[:, :, 0:129, 1],
                                  psv[:, :, OD0:OD0 + NOD], 1.0 / 17.0)
                nc.sync.dma_start(
                    out=bass.AP(out_ext, (c * HP + 1) * WP,
                                [[8 * WP, 8], [WP, NS], [1, WP]]),
                    in_=_s(o_t, 260)[:, :, 0:WP],
                )
    nc.finalize()
    return nc


_CACHE: dict = {}


def _get_nc():
    if "nc" not in _CACHE:
        _CACHE["nc"] = build_nc()
    return _CACHE["nc"]


def make_in_maps(x: np.ndarray):
    w8 = np.zeros((128, 8), ml_dtypes.bfloat16)
    w8[np.arange(128), np.arange(128) % 8] = 1.0
    return [
        {"x": np.ascontiguousarray(x[i]), "w8": w8}
        for i in range(N_CORES)
    ]


def kernel(**inputs) -> np.ndarray:
    x = np.ascontiguousarray(np.asarray(inputs["x"], dtype=np.float32))
    assert x.shape == (N_CORES, C, D, H, W), x.shape
    nc = _get_nc()
    res = run_bass_kernel_spmd(nc, make_in_maps(x), list(range(N_CORES)))
    return np.stack([res.results[i]["out"] for i in range(N_CORES)], axis=0)
